# revision 2
# baseline (speedup 1.0000x reference)
"""BiDenseGeneral (binarized dense) Trainium2 kernel.

Math (from the reference):
  w_bound[f] = max_d |W[d,f]| + eps32 ;  k_bin = sign01(W) * w_bound/2
  a_bound[r] = max_d |x[r,d]| + eps32 ;  x_bin = sign01(x) * a_bound/2
  out[r,f]   = sum_d x_bin[r,d] * k_bin[d,f]   (+ bias)
where sign01(v) = +1 if v >= 0 else -1.

Key identity: out = (S/4) * a_bound[r] * w_bound[f] with
  S/4 = sum_d (+-0.5)*(+-0.5), which is *exact* in fp8xfp8->fp32-PSUM
matmul arithmetic (products are +-0.25, partial sums are exact multiples
of 0.25 well below 2^24). So the matmul runs at full fp8 DoubleRow rate
with zero accuracy loss; the two rank-1 scales are applied on PSUM
eviction via a single fused scalar_tensor_tensor op.

Sharding: 4 row-groups x 2 feature-groups over 8 cores. Inputs are
pre-transposed on the host (layout choice only) so the contraction dim
lands on SBUF partitions with no on-device transposes of bulk data.
"""
import sys
import types

sys.path.insert(0, '/opt/trn_rl_repo')

import numpy as np

import concourse.bass as bass
import concourse.mybir as mybir
from concourse.tile import TileContext
from concourse.bass_utils import run_bass_kernel_spmd
from concourse.masks import make_identity

EPS32 = float(np.finfo(np.float32).eps)

# Problem shapes (hardcoded per contract)
B, S, DM, FF = 4, 4096, 2048, 8192
N_CORES = 8
RG, FG = 4, 2               # row-groups x feature-groups
R = (B * S) // RG           # 4096 rows per core
F = FF // FG                # 4096 features per core
D = DM                      # 2048 contraction
P = 128
ND = D // P                 # 16 d-tiles
NPAIR = ND // 2             # 8 DoubleRow K-pairs
NRB = R // P                # 32 r-blocks
NFB = F // 512              # 8 f-blocks (PSUM bank each)
NFC = F // P                # 32 f-chunks (colmax transposes)
HS = 2048                   # staging half width
NH = 2

f8 = mybir.dt.float8e4
f32 = mybir.dt.float32
A = mybir.AluOpType
AF = mybir.ActivationFunctionType


def _install_ntff_shim():
    """Register the NTFF profiling hook missing from this image (optional)."""
    if 'antenv.axon_hooks' in sys.modules:
        return
    try:
        mod = types.ModuleType('antenv.axon_hooks')
        _h = [None]
        mod.set_axon_ntff_profile_hook = lambda h: _h.__setitem__(0, h)
        mod.get_axon_ntff_profile_hook = lambda: _h[0]
        sys.modules['antenv.axon_hooks'] = mod
        import antenv
        antenv.axon_hooks = mod
        from trn_agent_boot.trn_boot import _ntff_profile_via_ctypes
        mod.set_axon_ntff_profile_hook(
            _ntff_profile_via_ctypes('/opt/axon/libaxon_pjrt.so'))
    except Exception:
        pass


def _split_sync_waits(nc, max_waits=1):
    """This walrus build rejects instructions with more than ~1 sem wait
    ("Too many sync wait commands"); hoist extras onto preceding NoOps."""
    n = 0
    for f in nc.m.functions:
        for blk in f.blocks:
            out = []
            for inst in blk.instructions:
                si = inst.sync_info
                waits = list(si.on_wait) if si and si.on_wait else []
                if len(waits) > max_waits:
                    keep = waits[-max_waits:]
                    for k, w in enumerate(waits[:-max_waits]):
                        nop = mybir.InstNoOp(
                            name=f"{inst.name}-wsplit{k}", ins=[], outs=[])
                        nop.engine = inst.engine
                        nop.sync_info = mybir.SyncInfo(on_wait=[w], on_update=[])
                        out.append(nop)
                        n += 1
                    si.on_wait = keep
                    inst.sync_info = si
                out.append(inst)
            blk.instructions = out
    return n


def _build_nc():
    nc = bass.Bass()
    xt_ext = nc.declare_dram_parameter("xt", [D, R], f32, isOutput=False)
    wt_ext = nc.declare_dram_parameter("wt", [D, F], f32, isOutput=False)
    out_ext = nc.declare_dram_parameter("out", [R, F], f32, isOutput=True)
    wb_scratch = nc.dram_tensor("wb_scratch", [F], f32)

    with TileContext(nc) as tc:
        with tc.tile_pool(name="stage", bufs=3) as stage, \
             tc.tile_pool(name="maxp", bufs=1) as maxpool, \
             tc.tile_pool(name="wrow", bufs=2) as wrowpool, \
             tc.tile_pool(name="outp", bufs=2) as outpool, \
             tc.tile_pool(name="pers", bufs=1) as pers, \
             tc.tile_pool(name="psum", bufs=8, space="PSUM") as psum_pool:

            x8 = pers.tile([P, ND, R], f8)
            w8 = pers.tile([P, ND, F], f8)
            wbh_rep = pers.tile([P, F], f32)
            abh = pers.tile([P, NRB], f32)
            ident = pers.tile([P, P], f32)
            ones1 = pers.tile([1, P], f32)
            wbT = pers.tile([P, NFC], f32)

            make_identity(nc, ident[:])
            nc.vector.memset(ones1[:], 1.0)

            # ---- W phase: sign + colmax partials ----
            wmaxp = maxpool.tile([P, F], f32, tag="maxp")
            for b in range(ND):
                for h in range(NH):
                    fr = slice(h * HS, (h + 1) * HS)
                    stg = stage.tile([P, HS], f32, tag="stage")
                    nc.sync.dma_start(out=stg[:], in_=wt_ext[b*P:(b+1)*P, fr])
                    nc.vector.tensor_scalar(
                        out=w8[:, b, fr], in0=stg[:], scalar1=0.0,
                        op0=A.is_ge, scalar2=0.5, op1=A.subtract)
                    if b == 0:
                        nc.scalar.activation(out=wmaxp[:, fr], in_=stg[:],
                                             func=AF.Abs)
                    else:
                        nc.scalar.activation(out=stg[:], in_=stg[:], func=AF.Abs)
                        nc.vector.tensor_tensor(out=wmaxp[:, fr], in0=stg[:],
                                                in1=wmaxp[:, fr], op=A.max)

            # colmax across partitions: PE-transpose chunks + free-dim reduce
            for c in range(NFC):
                pt = psum_pool.tile([P, 512], f32, tag="mm")
                nc.tensor.transpose(pt[:, 0:P], wmaxp[:, c*P:(c+1)*P], ident[:])
                nc.vector.reduce_max(out=wbT[:, c:c+1], in_=pt[:, 0:P],
                                     axis=mybir.AxisListType.X)
            # flatten wbT [P, NFC] -> DRAM scratch in f-order
            nc.sync.dma_start(out=wb_scratch[:].rearrange("(c p) -> p c", p=P),
                              in_=wbT[:, :])
            # broadcast w_bound to all partitions: K=1 matmul per f-block
            for fb in range(NFB):
                wrow = wrowpool.tile([1, 512], f32, tag="wrow")
                nc.sync.dma_start(out=wrow[:],
                                  in_=wb_scratch[fb*512:(fb+1)*512])
                pb = psum_pool.tile([P, 512], f32, tag="mm")
                nc.tensor.matmul(pb[:], lhsT=ones1[:], rhs=wrow[0:1, :],
                                 start=True, stop=True)
                nc.vector.tensor_scalar(
                    out=wbh_rep[:, fb*512:(fb+1)*512], in0=pb[:],
                    scalar1=EPS32, scalar2=None, op0=A.add)

            # ---- X phase: sign + rowmax partials ----
            xmaxp = maxpool.tile([P, R], f32, tag="maxp")
            for b in range(ND):
                for h in range(NH):
                    rr = slice(h * HS, (h + 1) * HS)
                    stg = stage.tile([P, HS], f32, tag="stage")
                    nc.sync.dma_start(out=stg[:], in_=xt_ext[b*P:(b+1)*P, rr])
                    nc.vector.tensor_scalar(
                        out=x8[:, b, rr], in0=stg[:], scalar1=0.0,
                        op0=A.is_ge, scalar2=0.5, op1=A.subtract)
                    if b == 0:
                        nc.scalar.activation(out=xmaxp[:, rr], in_=stg[:],
                                             func=AF.Abs)
                    else:
                        nc.scalar.activation(out=stg[:], in_=stg[:], func=AF.Abs)
                        nc.vector.tensor_tensor(out=xmaxp[:, rr], in0=stg[:],
                                                in1=xmaxp[:, rr], op=A.max)

            # rowmax across partitions -> abh[:, rb]
            for rb in range(NRB):
                pt = psum_pool.tile([P, 512], f32, tag="mm")
                nc.tensor.transpose(pt[:, 0:P], xmaxp[:, rb*P:(rb+1)*P], ident[:])
                nc.vector.reduce_max(out=abh[:, rb:rb+1], in_=pt[:, 0:P],
                                     axis=mybir.AxisListType.X)
            nc.vector.tensor_scalar(out=abh[:], in0=abh[:], scalar1=EPS32,
                                    scalar2=None, op0=A.add)

            # ---- main matmul loop ----
            for rb in range(NRB):
                outh = []
                for h in range(NH):
                    outh_t = outpool.tile([P, HS], f32, tag="outh")
                    outh.append(outh_t)
                pts = []
                for fb in range(NFB):
                    pt = psum_pool.tile([P, 512], f32, tag="mm")
                    pts.append(pt)
                for j in range(NPAIR):
                    for fb in range(NFB):
                        nc.tensor.matmul(
                            pts[fb][:],
                            lhsT=x8[:, 2*j:2*j+2, rb*P:(rb+1)*P],
                            rhs=w8[:, 2*j:2*j+2, fb*512:(fb+1)*512],
                            start=(j == 0), stop=(j == NPAIR - 1),
                            perf_mode=mybir.MatmulPerfMode.DoubleRow)
                for fb in range(NFB):
                    h = fb // (NFB // NH)
                    col = (fb % (NFB // NH)) * 512
                    nc.vector.scalar_tensor_tensor(
                        out=outh[h][:, col:col+512],
                        in0=pts[fb][:], scalar=abh[:, rb:rb+1],
                        in1=wbh_rep[:, fb*512:(fb+1)*512],
                        op0=A.mult, op1=A.mult)
                    if fb % (NFB // NH) == NFB // NH - 1:
                        nc.gpsimd.dma_start(
                            out=out_ext[rb*P:(rb+1)*P, h*HS:(h+1)*HS],
                            in_=outh[h][:])

    _split_sync_waits(nc)
    return nc


_NC_CACHE = [None]


def _get_nc():
    if _NC_CACHE[0] is None:
        _NC_CACHE[0] = _build_nc()
    return _NC_CACHE[0]


def _make_in_maps(x, w):
    xf = np.ascontiguousarray(x.reshape(B * S, DM))
    xT = np.ascontiguousarray(xf.T)         # [D, B*S]
    in_maps = []
    for c in range(N_CORES):
        rg, fg = divmod(c, FG)
        in_maps.append({
            "xt": np.ascontiguousarray(xT[:, rg*R:(rg+1)*R]),
            "wt": np.ascontiguousarray(w[:, fg*F:(fg+1)*F]),
        })
    return in_maps


def _assemble(results):
    out = np.empty((B * S, FF), np.float32)
    for c in range(N_CORES):
        rg, fg = divmod(c, FG)
        out[rg*R:(rg+1)*R, fg*F:(fg+1)*F] = results[c]["out"]
    return out.reshape(B, S, FF)


def run_device(x, w, trace=False):
    """Run the SPMD kernel; returns (out[B,S,FF], BassKernelResults)."""
    if trace:
        _install_ntff_shim()
    nc = _get_nc()
    in_maps = _make_in_maps(x, w)
    res = run_bass_kernel_spmd(nc, in_maps, core_ids=list(range(N_CORES)),
                               trace=trace)
    return _assemble(res.results), res


def kernel(inputs, kernel, bias):
    out, _ = run_device(np.asarray(inputs, dtype=np.float32),
                        np.asarray(kernel, dtype=np.float32))
    if bias is not None:
        bias = np.asarray(bias, dtype=np.float32)
        if np.any(bias):
            out = out + bias
    return out
